# revision 5
# baseline (speedup 1.0000x reference)
"""HRGNN message-passing kernel for 8 Trainium2 NeuronCores.

Strategy: shard edges by dst-node range (core k owns nodes [6250k,6250(k+1))
and every edge pointing into that range). Softmax segment stats then complete
locally per core — no collectives. Per-edge work on device:
  - z table (z|p|q) built once per core via matmuls (z = h@fc_w.T,
    p = z@aw0, q = z@aw1 folded into the same matmul pass)
  - dma_gather of z[src] rows (+p[src]) and (p,q)[dst_local]
  - e = leaky_relu(p[src]+q[dst]+r_h@c2) via matmul column + DVE/ACT
  - msg = exp(e)*(z[src]+r_h@fc_r_w.T), den rides as column 128
  - dma_scatter_add (SBUF parity mode) accumulates msg/den per node
  - out = relu(agg/den + h_own@(loop_w@fc_w).T)
src indices are split at 25024 into two halves so gather indices fit int16.
"""
import sys, os, time
sys.path.insert(0, '/opt/trn_rl_repo')
import numpy as np

import concourse.bass as bass
import concourse.bacc as bacc
import concourse.tile as tile
import concourse.mybir as mybir
from concourse import bass_utils

F32 = mybir.dt.float32
I16 = mybir.dt.int16
AF = mybir.ActivationFunctionType

N_CORES = 8
N, E, D = 50000, 800000, 128
NPC = N // N_CORES          # 6250 nodes per core
NPAD = 6400                 # padded own-node count (50*128)
SPLIT = 25024               # src-half split point
ROWS_A = 25088              # zxA rows: nodes [0, 25088)
OFF_B = 24960               # zxB base node
ROWS_B = 25088              # zxB rows: nodes [24960, 50048)
HCOLS = 50048               # padded node count (391*128)
CHUNK = 128                 # edges per matmul chunk
NCH = 17                    # chunks per edge tile
T = CHUNK * NCH             # 2176 edges per tile
NT = 26                     # tiles per half
BPAD = T * NT               # 56576 padded edges per half
AROWS = 8704                # agg_ext rows (6400 real + dummy pad rows)
ZBIG = 3584                 # z-phase streaming chunk (28*128); 25088 = 7*3584
TPR = 64                    # scatter sbuf_tokens_per_rank
GRP = 50                    # scatter groups = ceil(6400/64/2)
ELEM = 130                  # scatter row: 128 msg + den + pad
NEG_SLOPE = 0.01

_CACHE: dict = {}


def build_program():
    nc = bacc.Bacc("TRN2", target_bir_lowering=False, debug=False,
                   num_devices=N_CORES)

    # ---- I/O ----
    hT = nc.dram_tensor("hT", [D, HCOLS], F32, kind="ExternalInput")
    hT_own = nc.dram_tensor("hT_own", [D, NPAD], F32, kind="ExternalInput")
    rhT = {h: nc.dram_tensor(f"rhT{h}", [D, BPAD], F32, kind="ExternalInput")
           for h in "AB"}
    gi = {h: nc.dram_tensor(f"gi{h}", [128, BPAD // 16], I16, kind="ExternalInput")
          for h in "AB"}
    di = {h: nc.dram_tensor(f"di{h}", [128, BPAD // 16], I16, kind="ExternalInput")
          for h in "AB"}
    ds = {h: nc.dram_tensor(f"ds{h}", [128, BPAD // 16], I16, kind="ExternalInput")
          for h in "AB"}
    fcwT_c01 = nc.dram_tensor("fcwT_c01", [D, 130], F32, kind="ExternalInput")
    w2T_c01 = nc.dram_tensor("w2T_c01", [D, 130], F32, kind="ExternalInput")
    fcrwT_c2 = nc.dram_tensor("fcrwT_c2", [D, 129], F32, kind="ExternalInput")
    out = nc.dram_tensor("out", [NPAD, D], F32, kind="ExternalOutput")

    # ---- DRAM scratch ----
    zx = {"A": nc.dram_tensor("zxA", [ROWS_A, 192], F32, kind="Internal"),
          "B": nc.dram_tensor("zxB", [ROWS_B, 192], F32, kind="Internal")}
    qptab = nc.dram_tensor("qptab", [NPAD, 64], F32, kind="Internal")
    zloop_scr = nc.dram_tensor("zloop_scr", [NPAD, D], F32, kind="Internal")
    agg_ext = nc.dram_tensor("agg_ext", [AROWS, 192], F32, kind="Internal")

    with tile.TileContext(nc) as tc:
        with tc.tile_pool(name="wts", bufs=1) as wp, \
             tc.tile_pool(name="stream", bufs=2) as sp, \
             tc.tile_pool(name="gath", bufs=2) as gp, \
             tc.tile_pool(name="msgp", bufs=2) as mp, \
             tc.tile_pool(name="small", bufs=2) as ep, \
             tc.tile_pool(name="aggp", bufs=1) as ap_, \
             tc.tile_pool(name="idxp", bufs=1) as ip, \
             tc.tile_pool(name="zps", bufs=2, space="PSUM") as zpp, \
             tc.tile_pool(name="eps", bufs=2, space="PSUM") as epp, \
             tc.tile_pool(name="mps", bufs=4, space="PSUM") as mpp:

            # resident weights
            fcw_sb = wp.tile([D, 130], F32)
            nc.sync.dma_start(fcw_sb[:], fcwT_c01.ap())
            w2_sb = wp.tile([D, 130], F32)
            nc.sync.dma_start(w2_sb[:], w2T_c01.ap())
            fcr_sb = wp.tile([D, 129], F32)
            nc.sync.dma_start(fcr_sb[:], fcrwT_c2.ap())

            # zero agg_ext scratch (Internal DRAM is uninitialized)
            zt = ap_.tile([D, 17, 192], F32)
            nc.vector.memset(zt[:], 0.0)
            agg_v = agg_ext.ap().rearrange("(p a) b -> p a b", p=128)
            for a0 in range(0, AROWS // 128, 17):
                nc.sync.dma_start(agg_v[:, a0:a0 + 17, :], zt[:])

            # ---- phase Z: build zx tables ----
            for tab, base in (("A", 0), ("B", OFF_B)):
                for big in range(ROWS_A // ZBIG):
                    hs = sp.tile([D, ZBIG], F32, tag="stream")
                    nc.sync.dma_start(hs[:], hT.ap()[:, base + big * ZBIG:
                                                     base + (big + 1) * ZBIG])
                    for c in range(ZBIG // CHUNK):
                        zp = zpp.tile([D, 130], F32)
                        nc.tensor.matmul(zp[:], hs[:, c * CHUNK:(c + 1) * CHUNK],
                                         fcw_sb[:], start=True, stop=True)
                        zo = ep.tile([D, 130], F32, tag="zout")
                        nc.scalar.copy(zo[:], zp[:])
                        r0 = big * ZBIG + c * CHUNK
                        nc.sync.dma_start(zx[tab].ap()[r0:r0 + CHUNK, 0:130], zo[:])

            # ---- phase OWN: zloop + qptab ----
            for big in range(2):
                hs = sp.tile([D, 3200], F32, tag="stream")
                nc.sync.dma_start(hs[:], hT_own.ap()[:, big * 3200:(big + 1) * 3200])
                for c in range(25):
                    zp = zpp.tile([D, 130], F32)
                    nc.tensor.matmul(zp[:], hs[:, c * CHUNK:(c + 1) * CHUNK],
                                     w2_sb[:], start=True, stop=True)
                    zo = ep.tile([D, 130], F32, tag="zout")
                    nc.scalar.copy(zo[:], zp[:])
                    r0 = big * 3200 + c * CHUNK
                    nc.sync.dma_start(zloop_scr.ap()[r0:r0 + CHUNK, :],
                                      zo[:, 0:128])
                    nc.sync.dma_start(qptab.ap()[r0:r0 + CHUNK, 0:2],
                                      zo[:, 128:130])

            # ---- phase E/MSG per half ----
            for h in "AB":
                gis = ip.tile([128, BPAD // 16], I16, tag=f"gi{h}")
                nc.sync.dma_start(gis[:], gi[h].ap())
                dis = ip.tile([128, BPAD // 16], I16, tag=f"di{h}")
                nc.sync.dma_start(dis[:], di[h].ap())
                dss = ip.tile([128, BPAD // 16], I16, tag=f"ds{h}")
                nc.sync.dma_start(dss[:], ds[h].ap())
                for t in range(NT):
                    rs = sp.tile([D, T], F32, tag="stream")
                    nc.sync.dma_start(rs[:], rhT[h].ap()[:, t * T:(t + 1) * T])
                    gA = gp.tile([D, NCH, 192], F32, tag="gA")
                    nc.gpsimd.dma_gather(gA[:], zx[h].ap(),
                                         gis[:, t * (T // 16):(t + 1) * (T // 16)],
                                         T, T, 192, single_packet=False)
                    gC = gp.tile([D, NCH, 64], F32, tag="gC")
                    nc.gpsimd.dma_gather(gC[:], qptab.ap(),
                                         dis[:, t * (T // 16):(t + 1) * (T // 16)],
                                         T, T, 64, single_packet=False)
                    # passE: er columns
                    pe = epp.tile([D, NCH], F32)
                    for c in range(NCH):
                        nc.tensor.matmul(pe[:, c:c + 1],
                                         rs[:, c * CHUNK:(c + 1) * CHUNK],
                                         fcr_sb[:, 128:129], start=True, stop=True)
                    # batched e/ex
                    s = ep.tile([D, NCH], F32, tag="s")
                    nc.vector.tensor_add(s[:], pe[:], gA[:, :, 128])
                    nc.vector.tensor_add(s[:], s[:], gC[:, :, 1])
                    e = ep.tile([D, NCH], F32, tag="e")
                    nc.vector.scalar_tensor_tensor(
                        e[:], s[:], NEG_SLOPE, s[:],
                        op0=mybir.AluOpType.mult, op1=mybir.AluOpType.max)
                    ex = ep.tile([D, NCH], F32, tag="ex")
                    nc.scalar.activation(ex[:], e[:], AF.Exp)
                    msg = mp.tile([D, NCH, ELEM], F32, tag="msg")
                    nc.vector.tensor_copy(msg[:, :, 128], ex[:])
                    # passMsg
                    for c in range(NCH):
                        pm = mpp.tile([D, D], F32)
                        nc.tensor.matmul(pm[:], rs[:, c * CHUNK:(c + 1) * CHUNK],
                                         fcr_sb[:, 0:128], start=True, stop=True)
                        tm = ep.tile([D, D], F32, tag="tm")
                        nc.scalar.mul(tm[:], pm[:], ex[:, c:c + 1])
                        nc.vector.scalar_tensor_tensor(
                            msg[:, c, 0:128], gA[:, c, 0:128], ex[:, c:c + 1],
                            tm[:], op0=mybir.AluOpType.mult,
                            op1=mybir.AluOpType.add)
                    nc.gpsimd.dma_scatter_add(
                        agg_ext.ap()[:, 0:ELEM], msg[:],
                        dss[:, t * (T // 16):(t + 1) * (T // 16)],
                        T, T, ELEM, elem_step=192, single_packet=False)

            # ---- final ----
            for t in range(NPAD // CHUNK):
                ag = ep.tile([D, ELEM], F32, tag="ag")
                nc.sync.dma_start(ag[:], agg_ext.ap()[t * CHUNK:(t + 1) * CHUNK, 0:ELEM])
                zl = ep.tile([D, D], F32, tag="zl")
                nc.sync.dma_start(zl[:], zloop_scr.ap()[t * CHUNK:(t + 1) * CHUNK, :])
                dn = ep.tile([D, 1], F32, tag="dn")
                nc.vector.tensor_scalar_max(dn[:], ag[:, 128:129], 1e-30)
                dr = ep.tile([D, 1], F32, tag="dr")
                nc.vector.reciprocal(dr[:], dn[:])
                o1 = ep.tile([D, D], F32, tag="o1")
                nc.vector.scalar_tensor_tensor(
                    o1[:], ag[:, 0:128], dr[:], zl[:],
                    op0=mybir.AluOpType.mult, op1=mybir.AluOpType.add)
                o2 = ep.tile([D, D], F32, tag="o2")
                nc.scalar.activation(o2[:], o1[:], AF.Relu)
                nc.sync.dma_start(out.ap()[t * CHUNK:(t + 1) * CHUNK, :], o2[:])

    nc.compile()
    return nc


def _wrap_idx(idx):
    return np.tile(np.ascontiguousarray(idx.reshape(-1, 16).T), (8, 1))


def _pack_tiles(dloc):
    """Assign each edge a slot in [0,BPAD) so every tile of T slots has
    distinct dst. Returns positions array aligned with dst-sorted order."""
    import heapq
    n = len(dloc)
    order = np.argsort(dloc, kind="stable")
    _, starts, counts = np.unique(dloc[order], return_index=True,
                                  return_counts=True)
    heap = [(0, t) for t in range(NT)]
    heapq.heapify(heap)
    assign = np.empty(n, np.int32)
    for s0, cnt in zip(starts, counts):
        assert cnt <= NT, f"node degree {cnt} exceeds NT={NT}"
        picked = [heapq.heappop(heap) for _ in range(cnt)]
        for j, (f, tt) in enumerate(picked):
            assign[s0 + j] = tt
            heapq.heappush(heap, (f + 1, tt))
    fills = np.bincount(assign, minlength=NT)
    assert fills.max() <= T, fills.max()
    o2 = np.argsort(assign, kind="stable")
    starts2 = np.zeros(NT, np.int64)
    starts2[1:] = np.cumsum(fills)[:-1]
    slot = np.arange(n) - starts2[assign[o2]]
    pos = assign[o2] * T + slot
    return order[o2], pos


def prepare_inputs(h, r_h, fc_w, fc_r_w, attn_w, loop_w, src, dst):
    h = np.asarray(h, np.float32); r_h = np.asarray(r_h, np.float32)
    fc_w = np.asarray(fc_w, np.float32); fc_r_w = np.asarray(fc_r_w, np.float32)
    attn_w = np.asarray(attn_w, np.float32); loop_w = np.asarray(loop_w, np.float32)
    src = np.asarray(src); dst = np.asarray(dst)
    aw = attn_w.reshape(3, D)
    c0 = fc_w.T @ aw[0]; c1 = fc_w.T @ aw[1]; c2 = fc_r_w.T @ aw[2]
    fcwT_c01 = np.ascontiguousarray(
        np.concatenate([fc_w.T, c0[:, None], c1[:, None]], 1), np.float32)
    w2T = fc_w.T @ loop_w          # zloop = z @ loop_w = h @ (fc_w.T @ loop_w)
    w2T_c01 = np.ascontiguousarray(
        np.concatenate([w2T, c0[:, None], c1[:, None]], 1), np.float32)
    fcrwT_c2 = np.ascontiguousarray(
        np.concatenate([fc_r_w.T, c2[:, None]], 1), np.float32)
    hT = np.zeros((D, HCOLS), np.float32)
    hT[:, :N] = h.T

    in_maps = []
    for k in range(N_CORES):
        m = (dst >= NPC * k) & (dst < NPC * (k + 1))
        eids = np.nonzero(m)[0]
        s_k, d_k = src[eids], dst[eids] - NPC * k
        im = {"hT": hT, "fcwT_c01": fcwT_c01, "w2T_c01": w2T_c01,
              "fcrwT_c2": fcrwT_c2}
        ho = np.zeros((D, NPAD), np.float32)
        ho[:, :NPC] = h[NPC * k:NPC * (k + 1)].T
        im["hT_own"] = ho
        for hn, sel, ioff in (("A", s_k < SPLIT, 0), ("B", s_k >= SPLIT, OFF_B)):
            ii = np.nonzero(sel)[0]
            n = len(ii)
            assert n <= BPAD, (k, hn, n)
            eord, pos = _pack_tiles(d_k[ii])
            eg = eids[ii[eord]]
            rt = np.zeros((D, BPAD), np.float32)
            rt[:, pos] = r_h[eg].T
            gidx = np.zeros(BPAD, np.int16)
            gidx[pos] = (src[eg] - ioff).astype(np.int16)
            didx = np.zeros(BPAD, np.int16)
            didx[pos] = d_k[ii[eord]].astype(np.int16)
            dsx = (NPAD + np.arange(BPAD) % T).astype(np.int16)
            dsx[pos] = d_k[ii[eord]].astype(np.int16)
            im[f"rhT{hn}"] = rt
            im[f"gi{hn}"] = _wrap_idx(gidx)
            im[f"di{hn}"] = _wrap_idx(didx)
            im[f"ds{hn}"] = _wrap_idx(dsx)
        in_maps.append(im)
    return in_maps


def get_nc():
    if "nc" not in _CACHE:
        _CACHE["nc"] = build_program()
    return _CACHE["nc"]


def run_device(in_maps):
    nc = get_nc()
    res = bass_utils.run_bass_kernel_spmd(nc, in_maps,
                                          core_ids=list(range(N_CORES)))
    return res.results


def kernel(**inputs):
    in_maps = prepare_inputs(**inputs)
    results = run_device(in_maps)
    out = np.empty((N, D), np.float32)
    for k in range(N_CORES):
        out[NPC * k:NPC * (k + 1)] = results[k]["out"][:NPC]
    return out
